# revision 49
# baseline (speedup 1.0000x reference)
"""Trainium2 Bass kernel for nn_CorollaryResonanceBank.

Pure data-parallel over batch: 8 cores x 32 batch rows.

Per core (v3 — overlapped scan, 3-engine pooling):
  Phase 1 (DMA-bound ~135us): stream receive/transmit spikes per batch
    pair. Window-pool T=2048 -> 128 bins: pairs 0-6 and 15 via DVE
    tensor_reduce; pairs 7-14 via one GpSimd halving stage (2048->1024)
    finished on the PE as 8 accumulating stride-8 matmuls per tensor
    (exact fp32, conv weights + trace coefs folded as always).
    Normalization is batched per lane on GpSimd (abs-max on DVE into
    m_all, then one chain + 16 J-writes), writing J = drive/thr.
  Scan: resonate-and-fire in negated state space (S^ = -s/thr) so the
    reset is one fused op: S^ = (S^' is_lt -1) add S^'. Spikes are
    recovered at the end as sum(S^) - sum(S^') (exact: each spike
    contributes exactly 1.0). States are packed (128 partitions, 2
    free) = (8 batch-groups x 16 resonators, 2 batches) to minimize
    per-op cost. Lane 1 (b 0:16) runs on DVE under the remaining DMA;
    lane 2 (b 16:32) tails with its u-op on GpSimd (3 DVE ops/step).
"""

import numpy as np

B, C, T, R, TB = 256, 64, 2048, 16, 128
W = T // TB            # 16 samples per time bin
NCORES = 8
BS = B // NCORES       # 32 batch rows per core
NPAIR = BS // 2        # 16 batch pairs per core

# pooling engine per pair slot: "dve" tensor_reduce, or "gph" = GpSimd
# halving + PE matmul finish. Pairs 0-7 on DVE so lane-1's norm is gated
# only by DVE's own (fast) reduces; pairs 8-15 on GpSimd+PE so DVE is
# free for the lane-1 scan.
POOL_ENG = ["dve"] * 8 + ["gph"] * 7 + ["dve"]
LANE1_CHUNK = 16        # lane-1 scan steps emitted between pairs 8..15

_runner = None


def _softplus(x):
    return np.log1p(np.exp(np.float64(x)))


def _sigmoid(x):
    return 1.0 / (1.0 + np.exp(-np.float64(x)))


def _build_consts(conv_w, conv_b, frequency, decay, threshold,
                  transmit_weight, receive_weight):
    conv_w = np.float64(conv_w)
    conv_b = np.float64(conv_b)
    sp_r = _softplus(receive_weight)
    sp_t = _softplus(transmit_weight)
    freq = 0.02 + 0.18 * _sigmoid(frequency)
    dec = 0.8 + 0.18 * _sigmoid(decay)
    thr = 0.35 + 0.75 * _sigmoid(threshold)
    chan = np.linspace(-1.0, 1.0, C)

    # Per-row coefficients for the 3 trace channels (sd, az, el), with the
    # 1/W window-mean folded in (exact: power-of-two scaling).
    coefR = np.zeros((2 * C, 3))
    coefR[:, 0] = 0.5 * sp_r / W
    coefR[0:C, 1] = 1.0 / W
    coefR[C:, 1] = -1.0 / W
    coefR[0:C, 2] = chan / W
    coefR[C:, 2] = chan / W
    coefT = np.zeros((C, 3))
    coefT[:, 0] = -sp_t / W

    def slot(coef):
        # lhsT block (K, 35): cols 0-15 drive (conv folded), 32-34 traces
        out = np.zeros((coef.shape[0], 35))
        out[:, 0:16] = coef @ conv_w.T
        out[:, 32:35] = coef
        return out

    wr = slot(coefR)                      # (128, 35) both batch slots
    wt = np.zeros((128, 99))
    wt[0:C, 0:35] = slot(coefT)           # pair's even batch -> rows 0-98
    wt[C:, 64:99] = slot(coefT)

    invthr = 1.0 / np.float32(thr)        # fp32 reciprocal of fp32 thr
    f32 = np.float32
    ff = freq.astype(f32)
    dd = dec.astype(f32)
    # second-order scan coefficients (fp32, matching the verified numpy
    # reference ordering): a = (1+dec)-f^2, c1 = 1-f^2
    a_c = ((f32(1.0) + dd).astype(f32) - (ff * ff).astype(f32)).astype(f32)
    c1_c = (f32(1.0) - (ff * ff).astype(f32)).astype(f32)
    # scan scalars per partition (16 r replicated over 8 groups):
    # col 0 dec, 1 f, 2 -f, 3 -1, 4 a, 5 c1, 6 -dec, 7 pad,
    # 8:24 invthr (x16 for the lane divide)
    scal16 = np.concatenate(
        [np.stack([dd, ff, -ff, -np.ones(R, f32), a_c, c1_c,
                   (-dd).astype(f32), np.zeros(R, f32)], axis=1),
         np.repeat(invthr[:, None], 16, axis=1)], axis=1)  # (16, 24)
    scal = np.tile(scal16, (8, 1))                          # (128, 24)
    # J-writes produce nfJ = -f*J directly: bias = -f * conv_b / thr
    biasn = np.zeros((128, 1), f32)
    bval = ((-ff) * (conv_b.astype(f32) * invthr).astype(f32)).astype(f32)
    biasn[0:16, 0] = bval
    biasn[64:80, 0] = bval
    return (wr.astype(np.float32), wt.astype(np.float32),
            scal.astype(np.float32), biasn.astype(np.float32))


def _build_nc():
    import concourse.bass as bass
    import concourse.tile as tile
    from concourse import bacc, mybir, bass_isa

    f32 = mybir.dt.float32
    Alu = mybir.AluOpType
    X = mybir.AxisListType.X

    nc = bacc.Bacc("TRN2")
    rcv = nc.dram_tensor("receive", [BS, 2, C, T], f32, kind="ExternalInput").ap()
    tms = nc.dram_tensor("transmit", [BS, C, T], f32, kind="ExternalInput").ap()
    wr_d = nc.dram_tensor("wr", [128, 35], f32, kind="ExternalInput").ap()
    wt_d = nc.dram_tensor("wt", [128, 99], f32, kind="ExternalInput").ap()
    scal_d = nc.dram_tensor("scal", [128, 24], f32, kind="ExternalInput").ap()
    biasn_d = nc.dram_tensor("biasn", [128, 1], f32, kind="ExternalInput").ap()
    # out[lane, r, b_local] = pooled spike rate for batch lane*16+b_local
    out_d = nc.dram_tensor("out", [2, 16, 16], f32, kind="ExternalOutput").ap()

    rcv_v = rcv.rearrange("b i c t -> b (i c) t")              # (32, 128, 2048)
    tm_v = tms.rearrange("(p two) c t -> p (two c) t", two=2)  # (16, 128, 2048)

    with tile.TileContext(nc) as tc:
        with (
            tc.tile_pool(name="io", bufs=4) as io,
            tc.tile_pool(name="pp", bufs=3) as ppool,
            tc.tile_pool(name="small", bufs=3) as small,
            tc.tile_pool(name="scan", bufs=4) as scanp,
            tc.tile_pool(name="persist", bufs=1) as persist,
            tc.tile_pool(name="psum", bufs=8, space="PSUM") as psum,
        ):
            wr_sb = persist.tile([128, 35], f32)
            nc.sync.dma_start(wr_sb[:], wr_d[:])
            wt_sb = persist.tile([128, 99], f32)
            nc.sync.dma_start(wt_sb[:], wt_d[:])
            scal_sb = persist.tile([128, 24], f32)
            nc.sync.dma_start(scal_sb[:], scal_d[:])
            biasn_sb = persist.tile([128, 1], f32)
            nc.sync.dma_start(biasn_sb[:], biasn_d[:])

            m1_s = scal_sb[0:16, 3:4]     # -1
            a_s = scal_sb[0:16, 4:5]      # a = (1+dec)-f^2
            c1_s = scal_sb[0:16, 5:6]     # c1 = 1-f^2
            nd_s = scal_sb[0:16, 6:7]     # -dec
            nf16 = scal_sb[0:16, 2:3]     # -f (folded into srep)
            ithr16 = scal_sb[0:16, 8:24]  # invthr replicated x16

            # per-lane scan state (16 r partitions, TB steps, 16 batches):
            # all writes at partition base 0 (engine partition bases must
            # be 32-aligned). Jl holds nfJ = -f*drive/thr; S1l = S^'
            # history; SPl = spikes.
            lanes = []
            for li in range(2):
                Jl = persist.tile([16, TB, 16], f32, tag=f"J{li}")
                S1l = persist.tile([16, TB, 16], f32, tag=f"S1{li}")
                SPl = persist.tile([16, TB, 16], f32, tag=f"SP{li}")
                lanes.append((Jl, S1l, SPl))
            zrow = persist.tile([16, 16], f32)
            nc.vector.memset(zrow[:], 0.0)
            zrow2 = persist.tile([16, 2, 16], f32)
            nc.vector.memset(zrow2[:], 0.0)
            m_all = persist.tile([3, BS], f32)

            def do_pool(p):
                rv0 = io.tile([128, T], f32, tag="rv0")
                nc.sync.dma_start(rv0[:], rcv_v[2 * p])
                rv1 = io.tile([128, T], f32, tag="rv1")
                nc.sync.dma_start(rv1[:], rcv_v[2 * p + 1])
                tm = io.tile([128, T], f32, tag="tm")
                nc.sync.dma_start(tm[:], tm_v[p])

                ps = psum.tile([99, TB], f32)
                if POOL_ENG[p] == "dve":
                    def window_pool(big, tag):
                        outp = ppool.tile([128, TB], f32, tag=tag)
                        nc.vector.tensor_reduce(
                            out=outp[:],
                            in_=big.rearrange("p (w q) -> p w q", q=W),
                            axis=X, op=Alu.add)
                        return outp

                    rv0p = window_pool(rv0, "rv0p")
                    rv1p = window_pool(rv1, "rv1p")
                    tmp = window_pool(tm, "tmp")
                    nc.tensor.matmul(ps[0:35, :], wr_sb[:], rv0p[:],
                                     start=True, stop=False,
                                     skip_group_check=True)
                    nc.tensor.matmul(ps[64:99, :], wr_sb[:], rv1p[:],
                                     start=True, stop=False,
                                     skip_group_check=True)
                    nc.tensor.matmul(ps[0:99, :], wt_sb[:], tmp[:],
                                     start=False, stop=True,
                                     skip_group_check=True)
                else:
                    # GpSimd halving stages (quartered ops so the 4-deep
                    # exec queue never commits >~0.6us ahead), then the PE
                    # finishes the window-sum as accumulating strided
                    # matmuls. "gph2" halves twice (PE 4 mm/tensor),
                    # "gph" once (PE 8 mm/tensor).
                    stages = 2 if POOL_ENG[p] == "gph2" else 1
                    red = W >> stages

                    def halve(big, tag):
                        cur = big
                        n = T
                        for s in range(stages):
                            n //= 2
                            h = ppool.tile([128, n], f32, tag=f"{tag}{s}")
                            pr = cur.rearrange("p (x two) -> p x two", two=2)
                            for q in range(4):
                                a, b = q * (n // 4), (q + 1) * (n // 4)
                                nc.gpsimd.tensor_add(h[:, a:b],
                                                     pr[:, a:b, 0],
                                                     pr[:, a:b, 1])
                            cur = h
                        return cur.rearrange("p (w q) -> p w q", q=red)

                    rv0h = halve(rv0, "rv0h")
                    rv1h = halve(rv1, "rv1h")
                    tmh = halve(tm, "tmh")
                    for j in range(red):
                        nc.tensor.matmul(ps[0:35, :], wr_sb[:], rv0h[:, :, j],
                                         start=(j == 0), stop=False,
                                         skip_group_check=True)
                    for j in range(red):
                        nc.tensor.matmul(ps[64:99, :], wr_sb[:], rv1h[:, :, j],
                                         start=(j == 0), stop=False,
                                         skip_group_check=True)
                    for j in range(red):
                        nc.tensor.matmul(ps[0:99, :], wt_sb[:], tmh[:, :, j],
                                         start=False, stop=(j == red - 1),
                                         skip_group_check=True)
                return ps

            def do_absmax(p, ps):
                # abs-max is a free-axis reduce -> DVE only
                nc.vector.tensor_reduce(
                    out=m_all[:, 2 * p:2 * p + 1], in_=ps[32:35, :], axis=X,
                    op=Alu.max, apply_absolute_value=True)
                nc.vector.tensor_reduce(
                    out=m_all[:, 2 * p + 1:2 * p + 2], in_=ps[96:99, :],
                    axis=X, op=Alu.max, apply_absolute_value=True)

            def lane_norm(lane, pss, c0=0, c1=16):
                # batched normalization chain (GpSimd head, ACT J-writes)
                # over lane-local batch columns [c0, c1)
                bs = lane * 16
                w = c1 - c0
                Jl = lanes[lane][0]
                mr = small.tile([3, w], f32, tag=f"mr{lane}{c0}")
                nc.gpsimd.partition_all_reduce(
                    mr[:], m_all[:, bs + c0:bs + c1], channels=3,
                    reduce_op=bass_isa.ReduceOp.max)
                srow = small.tile([1, w], f32, tag=f"srow{lane}{c0}")
                nc.vector.tensor_scalar(out=srow[:], in0=mr[0:1, :],
                                        scalar1=1.0, scalar2=None, op0=Alu.max)
                srecip = small.tile([1, w], f32, tag=f"srecip{lane}{c0}")
                nc.vector.reciprocal(out=srecip[:], in_=srow[:])
                sb16 = small.tile([16, w], f32, tag=f"sb16{lane}{c0}")
                nc.gpsimd.partition_broadcast(sb16[:], srecip[:])
                srep = small.tile([16, w], f32, tag=f"srep{lane}{c0}")
                nc.vector.tensor_tensor(srep[:], ithr16[:, 0:w], sb16[:],
                                        Alu.mult)
                srepn = small.tile([16, w], f32, tag=f"srepn{lane}{c0}")
                nc.vector.tensor_scalar(out=srepn[:], in0=srep[:],
                                        scalar1=nf16, scalar2=None,
                                        op0=Alu.mult)
                # J-writes on the otherwise-idle ACT engine:
                # nfJ = Identity(ps * (-f*srep) + (-f*conv_b/thr))
                for local in range(c0, c1):
                    b = bs + local
                    base = 64 if b % 2 else 0
                    ps = pss[b // 2]
                    nc.scalar.activation(
                        out=Jl[:, :, local],
                        in_=ps[base:base + 16, :],
                        func=mybir.ActivationFunctionType.Identity,
                        bias=biasn_sb[base:base + 16, :],
                        scale=srepn[:, local - c0:local - c0 + 1])

            def scan_lane(lane, t0=0, t1=TB):
                # second-order resonate-and-fire (negated space, depth 2):
                #   S^'_t = a*S^'_{t-1} - dec*S^'_{t-2}
                #           + c1*sp_{t-1} - dec*sp_{t-2} + nfJ_t
                #   sp_t  = (S^'_t < -1)
                # Z/Z2 read only t-2 state and J, so the critical chain is
                # S1_t -> H_{t+1} -> S1_{t+1} (and S1_t -> sp_t -> S1_{t+1}):
                # two dependent hops per step.
                Jl, S1l, SPl = lanes[lane]
                for t in range(t0, t1):
                    S1m1 = S1l[:, t - 1, :] if t >= 1 else zrow[:]
                    S1m2 = S1l[:, t - 2, :] if t >= 2 else zrow[:]
                    spm1 = SPl[:, t - 1, :] if t >= 1 else zrow[:]
                    spm2 = SPl[:, t - 2, :] if t >= 2 else zrow[:]
                    Z = scanp.tile([16, 16], f32, tag=f"z{lane}")
                    nc.vector.scalar_tensor_tensor(
                        out=Z[:], in0=S1m2, scalar=nd_s, in1=Jl[:, t, :],
                        op0=Alu.mult, op1=Alu.add)
                    Z2 = scanp.tile([16, 16], f32, tag=f"z2{lane}")
                    nc.vector.scalar_tensor_tensor(
                        out=Z2[:], in0=spm2, scalar=nd_s, in1=Z[:],
                        op0=Alu.mult, op1=Alu.add)
                    H = scanp.tile([16, 16], f32, tag=f"h{lane}")
                    nc.vector.scalar_tensor_tensor(
                        out=H[:], in0=S1m1, scalar=a_s, in1=Z2[:],
                        op0=Alu.mult, op1=Alu.add)
                    S1t = S1l[:, t, :]
                    nc.vector.scalar_tensor_tensor(
                        out=S1t, in0=spm1, scalar=c1_s, in1=H[:],
                        op0=Alu.mult, op1=Alu.add)
                    nc.vector.tensor_scalar(
                        out=SPl[:, t, :], in0=S1t, scalar1=m1_s,
                        scalar2=None, op0=Alu.is_lt)

            def lane_out(lane):
                SPl = lanes[lane][2]
                sums = small.tile([16, 16], f32, tag=f"sums_{lane}")
                nc.vector.tensor_reduce(
                    out=sums[:], in_=SPl.rearrange("p t b -> p b t"),
                    axis=X, op=Alu.add)
                ob = small.tile([16, 16], f32, tag=f"ob_{lane}")
                nc.vector.tensor_scalar(out=ob[:], in0=sums[:],
                                        scalar1=1.0 / TB, scalar2=None,
                                        op0=Alu.mult)
                nc.sync.dma_start(out_d[lane], ob[:])

            # ---- emission ----
            # Pairs 0-7: pool+matmul, then batched absmax + lane-1 norm.
            pss = {}
            for p in range(8):
                pss[p] = do_pool(p)
            for p in range(8):
                do_absmax(p, pss[p])
            lane_norm(0, pss)
            # Pairs 8-15: pool+matmul with lane-1 scan chunks interleaved
            # in the DVE queue. Each pair's absmax (2 parked ops, within
            # the scoreboard window) is staggered one chunk later so it
            # runs mid-lane-1 instead of serializing after it. Lane-2's
            # norm is split: pairs 8-14 as soon as their absmax is in,
            # pair 15 separately, so only 2 J-writes trail the last pair.
            t_emitted = 0
            for p in range(8, NPAIR):
                pss[p] = do_pool(p)
                if p > 8:
                    do_absmax(p - 1, pss[p - 1])
                scan_lane(0, t_emitted, t_emitted + LANE1_CHUNK)
                t_emitted += LANE1_CHUNK
            if t_emitted < TB:
                scan_lane(0, t_emitted, TB)
            lane_norm(1, pss, 0, 14)
            do_absmax(NPAIR - 1, pss[NPAIR - 1])
            lane_norm(1, pss, 14, 16)
            lane_out(0)
            scan_lane(1)
            lane_out(1)

    nc.compile()
    return nc


class _Runner:
    """Compiles the Bass program once and executes it via PJRT shard_map
    across the 8 NeuronCores (mirrors bass2jax.run_bass_via_pjrt, but
    keeps the jitted callable for cheap repeat calls)."""

    def __init__(self):
        import jax
        import numpy as _np
        from jax.sharding import Mesh, PartitionSpec
        from jax.experimental.shard_map import shard_map
        import concourse.mybir as mybir
        from concourse.bass2jax import (_bass_exec_p, install_neuronx_cc_hook,
                                        partition_id_tensor)

        install_neuronx_cc_hook()
        nc = _build_nc()
        self.nc = nc

        partition_name = (nc.partition_id_tensor.name
                          if nc.partition_id_tensor else None)
        in_names, out_names, out_avals, zero_outs = [], [], [], []
        for alloc in nc.m.functions[0].allocations:
            if not isinstance(alloc, mybir.MemoryLocationSet):
                continue
            name = alloc.memorylocations[0].name
            if alloc.kind == "ExternalInput":
                if name != partition_name:
                    in_names.append(name)
            elif alloc.kind == "ExternalOutput":
                out_names.append(name)
                shape = tuple(alloc.tensor_shape)
                dtype = mybir.dt.np(alloc.dtype)
                out_avals.append(jax.core.ShapedArray(shape, dtype))
                zero_outs.append(_np.zeros(shape, dtype))
        self.in_names = list(in_names)
        self.out_names = out_names
        n_params = len(in_names)
        all_in_names = in_names + out_names
        if partition_name is not None:
            all_in_names.append(partition_name)

        def _body(*args):
            operands = list(args)
            if partition_name is not None:
                operands.append(partition_id_tensor())
            outs = _bass_exec_p.bind(
                *operands,
                out_avals=tuple(out_avals),
                in_names=tuple(all_in_names),
                out_names=tuple(out_names),
                lowering_input_output_aliases=(),
                sim_require_finite=True,
                sim_require_nnan=True,
                nc=nc,
            )
            return tuple(outs)

        devices = jax.devices()[:NCORES]
        self.mesh = Mesh(np.asarray(devices), ("core",))
        in_specs = (PartitionSpec("core"),) * (n_params + len(out_names))
        out_specs = (PartitionSpec("core"),) * len(out_names)
        self.fn = jax.jit(shard_map(_body, mesh=self.mesh, in_specs=in_specs,
                                    out_specs=out_specs, check_rep=False),
                          keep_unused=True)
        self.zero_outs = zero_outs
        self.out_avals = out_avals

    def concat_inputs(self, per_core_maps):
        return [np.concatenate([m[name] for m in per_core_maps], axis=0)
                for name in self.in_names]

    def run(self, concat_in):
        concat_zeros = [np.zeros((NCORES * z.shape[0], *z.shape[1:]), z.dtype)
                        for z in self.zero_outs]
        out_arrs = self.fn(*concat_in, *concat_zeros)
        return [np.asarray(a) for a in out_arrs]


def _get_runner():
    global _runner
    if _runner is None:
        _runner = _Runner()
    return _runner


def kernel(**inputs):
    ts = np.ascontiguousarray(np.asarray(inputs["transmit_spikes"], np.float32))
    rs = np.ascontiguousarray(np.asarray(inputs["receive_spikes"], np.float32))
    wr, wt, scal, biasn = _build_consts(
        np.asarray(inputs["conv_w"]), np.asarray(inputs["conv_b"]),
        np.asarray(inputs["frequency"]), np.asarray(inputs["decay"]),
        np.asarray(inputs["threshold"]),
        np.asarray(inputs["transmit_weight"]),
        np.asarray(inputs["receive_weight"]))

    runner = _get_runner()
    per_core = []
    for cidx in range(NCORES):
        bsl = slice(cidx * BS, (cidx + 1) * BS)
        per_core.append({
            "receive": rs[bsl], "transmit": ts[bsl],
            "wr": wr, "wt": wt, "scal": scal, "biasn": biasn,
        })
    concat_in = runner.concat_inputs(per_core)
    outs = runner.run(concat_in)
    # output "out": (8*2, 16, 16) -> (core, lane, r, b_local);
    # batch b = 32*core + 16*lane + b_local, value row = r
    o = outs[0].reshape(NCORES, 2, 16, 16)
    full = np.transpose(o, (0, 1, 3, 2)).reshape(B, R)
    return np.ascontiguousarray(full.astype(np.float32))


# revision 62
# speedup vs baseline: 1.0179x; 1.0179x over previous
"""Trainium2 Bass kernel for nn_CorollaryResonanceBank.

Pure data-parallel over batch: 8 cores x 32 batch rows.

Per core (v4 — overlapped second-order scan, 3-engine pooling):
  Phase 1 (DMA-bound ~135us): stream receive/transmit spikes per batch
    pair. Window-pool T=2048 -> 128 bins: pairs 0-7 via DVE
    tensor_reduce; pairs 8-15 via GpSimd halving stage(s) finished on
    the PE as accumulating strided matmuls (exact fp32; conv weights +
    trace coefs host-folded into the same contraction as always).
    Normalization is batched per lane: abs-max on DVE into m_all, the
    short scale chain on GpSimd/DVE, then per-batch J-writes on the
    otherwise-idle ACT engine, producing nfJ = -f*drive/thr.
  Scan: resonate-and-fire rewritten as an exactly-equivalent
    second-order recurrence in the negated pre/post-reset states
    (verified spike-for-spike identical in fp32 against the reference):
      S1_t = c1*S^_{t-1} + dec*S1_{t-1} - dec*S^_{t-2} + nfJ_t
      S^_t = S1_t + (S1_t < -1)
    Four fused DVE ops per step with a dependent-hop depth of two.
    Spikes are recovered at the end as sum(S^) - sum(S1) (each spike
    contributes exactly 1.0). Lane 1 (b 0:16) runs overlapped under the
    remaining DMA as pure chunks (no other DVE ops interleaved, so the
    in-order queue never head-of-line blocks it); lane 2 (b 16:32)
    tails after the last pair's normalization.
"""

import numpy as np

B, C, T, R, TB = 256, 64, 2048, 16, 128
W = T // TB            # 16 samples per time bin
NCORES = 8
BS = B // NCORES       # 32 batch rows per core
NPAIR = BS // 2        # 16 batch pairs per core

# pooling engine per pair slot: "dve" tensor_reduce, or "gph" = GpSimd
# halving + PE matmul finish. Pairs 0-7 on DVE so lane-1's norm is gated
# only by DVE's own (fast) reduces; pairs 8-15 on GpSimd+PE so DVE is
# free for the lane-1 scan.
POOL_ENG = ["dve"] * 8 + ["gph"] * 7 + ["gph2"]
LANE1_CHUNK = 16        # lane-1 scan steps emitted between pairs 8..15

_runner = None


def _softplus(x):
    return np.log1p(np.exp(np.float64(x)))


def _sigmoid(x):
    return 1.0 / (1.0 + np.exp(-np.float64(x)))


def _build_consts(conv_w, conv_b, frequency, decay, threshold,
                  transmit_weight, receive_weight):
    conv_w = np.float64(conv_w)
    conv_b = np.float64(conv_b)
    sp_r = _softplus(receive_weight)
    sp_t = _softplus(transmit_weight)
    freq = 0.02 + 0.18 * _sigmoid(frequency)
    dec = 0.8 + 0.18 * _sigmoid(decay)
    thr = 0.35 + 0.75 * _sigmoid(threshold)
    chan = np.linspace(-1.0, 1.0, C)

    # Per-row coefficients for the 3 trace channels (sd, az, el), with the
    # 1/W window-mean folded in (exact: power-of-two scaling).
    coefR = np.zeros((2 * C, 3))
    coefR[:, 0] = 0.5 * sp_r / W
    coefR[0:C, 1] = 1.0 / W
    coefR[C:, 1] = -1.0 / W
    coefR[0:C, 2] = chan / W
    coefR[C:, 2] = chan / W
    coefT = np.zeros((C, 3))
    coefT[:, 0] = -sp_t / W

    def slot(coef):
        # lhsT block (K, 35): cols 0-15 drive (conv folded), 32-34 traces
        out = np.zeros((coef.shape[0], 35))
        out[:, 0:16] = coef @ conv_w.T
        out[:, 32:35] = coef
        return out

    wr = slot(coefR)                      # (128, 35) both batch slots
    wt = np.zeros((128, 99))
    wt[0:C, 0:35] = slot(coefT)           # pair's even batch -> rows 0-98
    wt[C:, 64:99] = slot(coefT)

    invthr = 1.0 / np.float32(thr)        # fp32 reciprocal of fp32 thr
    f32 = np.float32
    ff = freq.astype(f32)
    dd = dec.astype(f32)
    # second-order scan coefficients (fp32, matching the verified numpy
    # reference ordering): a = (1+dec)-f^2, c1 = 1-f^2
    a_c = ((f32(1.0) + dd).astype(f32) - (ff * ff).astype(f32)).astype(f32)
    c1_c = (f32(1.0) - (ff * ff).astype(f32)).astype(f32)
    # scan scalars per partition (16 r replicated over 8 groups):
    # col 0 dec, 1 f, 2 -f, 3 -1, 4 a, 5 c1, 6 -dec, 7 pad,
    # 8:24 invthr (x16 for the lane divide)
    scal16 = np.concatenate(
        [np.stack([dd, ff, -ff, -np.ones(R, f32), a_c, c1_c,
                   (-dd).astype(f32), np.zeros(R, f32)], axis=1),
         np.repeat(invthr[:, None], 16, axis=1)], axis=1)  # (16, 24)
    scal = np.tile(scal16, (8, 1))                          # (128, 24)
    # J-writes produce nfJ = -f*J directly: bias = -f * conv_b / thr
    biasn = np.zeros((128, 1), f32)
    bval = ((-ff) * (conv_b.astype(f32) * invthr).astype(f32)).astype(f32)
    biasn[0:16, 0] = bval
    biasn[64:80, 0] = bval
    return (wr.astype(np.float32), wt.astype(np.float32),
            scal.astype(np.float32), biasn.astype(np.float32))


def _build_nc():
    import concourse.bass as bass
    import concourse.tile as tile
    from concourse import bacc, mybir, bass_isa

    f32 = mybir.dt.float32
    Alu = mybir.AluOpType
    X = mybir.AxisListType.X

    nc = bacc.Bacc("TRN2")
    rcv = nc.dram_tensor("receive", [BS, 2, C, T], f32, kind="ExternalInput").ap()
    tms = nc.dram_tensor("transmit", [BS, C, T], f32, kind="ExternalInput").ap()
    wr_d = nc.dram_tensor("wr", [128, 35], f32, kind="ExternalInput").ap()
    wt_d = nc.dram_tensor("wt", [128, 99], f32, kind="ExternalInput").ap()
    scal_d = nc.dram_tensor("scal", [128, 24], f32, kind="ExternalInput").ap()
    biasn_d = nc.dram_tensor("biasn", [128, 1], f32, kind="ExternalInput").ap()
    # out[lane, r, b_local] = pooled spike rate for batch lane*16+b_local
    out_d = nc.dram_tensor("out", [2, 16, 16], f32, kind="ExternalOutput").ap()

    rcv_v = rcv.rearrange("b i c t -> b (i c) t")              # (32, 128, 2048)
    tm_v = tms.rearrange("(p two) c t -> p (two c) t", two=2)  # (16, 128, 2048)

    with tile.TileContext(nc) as tc:
        with (
            tc.tile_pool(name="io", bufs=4) as io,
            tc.tile_pool(name="pp", bufs=2) as ppool,
            tc.tile_pool(name="small", bufs=3) as small,
            tc.tile_pool(name="scan", bufs=4) as scanp,
            tc.tile_pool(name="persist", bufs=1) as persist,
            tc.tile_pool(name="psum", bufs=8, space="PSUM") as psum,
        ):
            wr_sb = persist.tile([128, 35], f32)
            nc.sync.dma_start(wr_sb[:], wr_d[:])
            wt_sb = persist.tile([128, 99], f32)
            nc.sync.dma_start(wt_sb[:], wt_d[:])
            scal_sb = persist.tile([128, 24], f32)
            nc.sync.dma_start(scal_sb[:], scal_d[:])
            biasn_sb = persist.tile([128, 1], f32)
            nc.sync.dma_start(biasn_sb[:], biasn_d[:])

            m1_s = scal_sb[0:16, 3:4]     # -1
            a_s = scal_sb[0:16, 4:5]      # a = (1+dec)-f^2
            c1_s = scal_sb[0:16, 5:6]     # c1 = 1-f^2
            nd_s = scal_sb[0:16, 6:7]     # -dec
            nf16 = scal_sb[0:16, 2:3]     # -f (folded into srep)
            ithr16 = scal_sb[0:16, 8:24]  # invthr replicated x16

            d_s = scal_sb[0:16, 0:1]      # dec

            # per-lane scan state (16 r partitions, TB steps, 16 batches):
            # all writes at partition base 0 (engine partition bases must
            # be 32-aligned). Jl holds nfJ = -f*drive/thr; S1l = S^'
            # (pre-reset) history; SHl = S^ (post-reset) history. Spikes
            # are recovered as sum(SH) - sum(S1).
            lanes = []
            for li in range(2):
                Jl = persist.tile([16, TB, 16], f32, tag=f"J{li}")
                S1l = persist.tile([16, TB, 16], f32, tag=f"S1{li}")
                SHl = persist.tile([16, TB, 16], f32, tag=f"SH{li}")
                lanes.append((Jl, S1l, SHl))
            zrow = persist.tile([16, 16], f32)
            nc.vector.memset(zrow[:], 0.0)
            zrow2 = persist.tile([16, 2, 16], f32)
            nc.vector.memset(zrow2[:], 0.0)
            m_all = persist.tile([3, BS], f32)

            def do_pool(p):
                rv0 = io.tile([128, T], f32, tag="rv0")
                nc.sync.dma_start(rv0[:], rcv_v[2 * p])
                rv1 = io.tile([128, T], f32, tag="rv1")
                nc.sync.dma_start(rv1[:], rcv_v[2 * p + 1])
                tm = io.tile([128, T], f32, tag="tm")
                nc.sync.dma_start(tm[:], tm_v[p])

                ps = psum.tile([99, TB], f32)
                if POOL_ENG[p] == "dve":
                    def window_pool(big, tag):
                        outp = ppool.tile([128, TB], f32, tag=tag)
                        nc.vector.tensor_reduce(
                            out=outp[:],
                            in_=big.rearrange("p (w q) -> p w q", q=W),
                            axis=X, op=Alu.add)
                        return outp

                    rv0p = window_pool(rv0, "rv0p")
                    rv1p = window_pool(rv1, "rv1p")
                    tmp = window_pool(tm, "tmp")
                    nc.tensor.matmul(ps[0:35, :], wr_sb[:], rv0p[:],
                                     start=True, stop=False,
                                     skip_group_check=True)
                    nc.tensor.matmul(ps[64:99, :], wr_sb[:], rv1p[:],
                                     start=True, stop=False,
                                     skip_group_check=True)
                    nc.tensor.matmul(ps[0:99, :], wt_sb[:], tmp[:],
                                     start=False, stop=True,
                                     skip_group_check=True)
                else:
                    # GpSimd halving stages (quartered ops so the 4-deep
                    # exec queue never commits >~0.6us ahead), then the PE
                    # finishes the window-sum as accumulating strided
                    # matmuls. "gph2" halves twice (PE 4 mm/tensor),
                    # "gph" once (PE 8 mm/tensor).
                    stages = 2 if POOL_ENG[p] == "gph2" else 1
                    red = W >> stages

                    def halve(big, tag):
                        cur = big
                        n = T
                        for s in range(stages):
                            n //= 2
                            h = ppool.tile([128, n], f32, tag=f"{tag}{s}")
                            pr = cur.rearrange("p (x two) -> p x two", two=2)
                            for q in range(4):
                                a, b = q * (n // 4), (q + 1) * (n // 4)
                                nc.gpsimd.tensor_add(h[:, a:b],
                                                     pr[:, a:b, 0],
                                                     pr[:, a:b, 1])
                            cur = h
                        return cur.rearrange("p (w q) -> p w q", q=red)

                    rv0h = halve(rv0, "rv0h")
                    rv1h = halve(rv1, "rv1h")
                    tmh = halve(tm, "tmh")
                    for j in range(red):
                        nc.tensor.matmul(ps[0:35, :], wr_sb[:], rv0h[:, :, j],
                                         start=(j == 0), stop=False,
                                         skip_group_check=True)
                    for j in range(red):
                        nc.tensor.matmul(ps[64:99, :], wr_sb[:], rv1h[:, :, j],
                                         start=(j == 0), stop=False,
                                         skip_group_check=True)
                    for j in range(red):
                        nc.tensor.matmul(ps[0:99, :], wt_sb[:], tmh[:, :, j],
                                         start=False, stop=(j == red - 1),
                                         skip_group_check=True)
                return ps

            def do_absmax(p, ps):
                # abs-max is a free-axis reduce -> DVE only
                nc.vector.tensor_reduce(
                    out=m_all[:, 2 * p:2 * p + 1], in_=ps[32:35, :], axis=X,
                    op=Alu.max, apply_absolute_value=True)
                nc.vector.tensor_reduce(
                    out=m_all[:, 2 * p + 1:2 * p + 2], in_=ps[96:99, :],
                    axis=X, op=Alu.max, apply_absolute_value=True)

            def lane_norm(lane, pss, c0=0, c1=16):
                # batched normalization chain (GpSimd head, ACT J-writes)
                # over lane-local batch columns [c0, c1)
                bs = lane * 16
                w = c1 - c0
                Jl = lanes[lane][0]
                mr = small.tile([3, w], f32, tag=f"mr{lane}{c0}")
                nc.gpsimd.partition_all_reduce(
                    mr[:], m_all[:, bs + c0:bs + c1], channels=3,
                    reduce_op=bass_isa.ReduceOp.max)
                srow = small.tile([1, w], f32, tag=f"srow{lane}{c0}")
                nc.vector.tensor_scalar(out=srow[:], in0=mr[0:1, :],
                                        scalar1=1.0, scalar2=None, op0=Alu.max)
                srecip = small.tile([1, w], f32, tag=f"srecip{lane}{c0}")
                nc.vector.reciprocal(out=srecip[:], in_=srow[:])
                sb16 = small.tile([16, w], f32, tag=f"sb16{lane}{c0}")
                nc.gpsimd.partition_broadcast(sb16[:], srecip[:])
                srep = small.tile([16, w], f32, tag=f"srep{lane}{c0}")
                nc.vector.tensor_tensor(srep[:], ithr16[:, 0:w], sb16[:],
                                        Alu.mult)
                srepn = small.tile([16, w], f32, tag=f"srepn{lane}{c0}")
                nc.vector.tensor_scalar(out=srepn[:], in0=srep[:],
                                        scalar1=nf16, scalar2=None,
                                        op0=Alu.mult)
                # J-writes on the otherwise-idle ACT engine:
                # nfJ = Identity(ps * (-f*srep) + (-f*conv_b/thr))
                for local in range(c0, c1):
                    b = bs + local
                    base = 64 if b % 2 else 0
                    ps = pss[b // 2]
                    nc.scalar.activation(
                        out=Jl[:, :, local],
                        in_=ps[base:base + 16, :],
                        func=mybir.ActivationFunctionType.Identity,
                        bias=biasn_sb[base:base + 16, :],
                        scale=srepn[:, local - c0:local - c0 + 1])

            def scan_lane(lane, t0=0, t1=TB):
                # second-order resonate-and-fire (negated space, depth 2):
                #   S^'_t = a*S^'_{t-1} - dec*S^'_{t-2}
                #           + c1*sp_{t-1} - dec*sp_{t-2} + nfJ_t
                #   sp_t  = (S^'_t < -1)
                # Four ops/step, two dependent hops (S^_{t-1} -> S1_t ->
                # S^_t; Z2 reads only t-2 state; K reads t-1 S1):
                #   S1_t = c1*S^_{t-1} + dec*S1_{t-1} - dec*S^_{t-2} + nfJ_t
                #   S^_t = S1_t + (S1_t < -1)
                Jl, S1l, SHl = lanes[lane]
                for t in range(t0, t1):
                    S1m1 = S1l[:, t - 1, :] if t >= 1 else zrow[:]
                    SHm1 = SHl[:, t - 1, :] if t >= 1 else zrow[:]
                    SHm2 = SHl[:, t - 2, :] if t >= 2 else zrow[:]
                    Z2 = scanp.tile([16, 16], f32, tag=f"z2{lane}")
                    nc.vector.scalar_tensor_tensor(
                        out=Z2[:], in0=SHm2, scalar=nd_s, in1=Jl[:, t, :],
                        op0=Alu.mult, op1=Alu.add)
                    K = scanp.tile([16, 16], f32, tag=f"k{lane}")
                    nc.vector.scalar_tensor_tensor(
                        out=K[:], in0=S1m1, scalar=d_s, in1=Z2[:],
                        op0=Alu.mult, op1=Alu.add)
                    S1t = S1l[:, t, :]
                    nc.vector.scalar_tensor_tensor(
                        out=S1t, in0=SHm1, scalar=c1_s, in1=K[:],
                        op0=Alu.mult, op1=Alu.add)
                    nc.vector.scalar_tensor_tensor(
                        out=SHl[:, t, :], in0=S1t, scalar=m1_s, in1=S1t,
                        op0=Alu.is_lt, op1=Alu.add)

            def lane_out(lane):
                Jl, S1l, SHl = lanes[lane]
                sum1 = small.tile([16, 16], f32, tag=f"sum1_{lane}")
                nc.vector.tensor_reduce(
                    out=sum1[:], in_=S1l.rearrange("p t b -> p b t"),
                    axis=X, op=Alu.add)
                sumh = small.tile([16, 16], f32, tag=f"sumh_{lane}")
                nc.vector.tensor_reduce(
                    out=sumh[:], in_=SHl.rearrange("p t b -> p b t"),
                    axis=X, op=Alu.add)
                dd = small.tile([16, 16], f32, tag=f"d_{lane}")
                nc.vector.tensor_tensor(dd[:], sumh[:], sum1[:], Alu.subtract)
                ob = small.tile([16, 16], f32, tag=f"ob_{lane}")
                nc.vector.tensor_scalar(out=ob[:], in0=dd[:],
                                        scalar1=1.0 / TB, scalar2=None,
                                        op0=Alu.mult)
                nc.sync.dma_start(out_d[lane], ob[:])

            # ---- emission ----
            # Pairs 0-7: pool+matmul, then batched absmax + lane-1 norm.
            pss = {}
            for p in range(8):
                pss[p] = do_pool(p)
            for p in range(8):
                do_absmax(p, pss[p])
            lane_norm(0, pss)
            # Pairs 8-15: pool+matmul with lane-1 scan chunks interleaved
            # in the DVE queue. Each pair's absmax (2 parked ops, within
            # the scoreboard window) is staggered one chunk later so it
            # runs mid-lane-1 instead of serializing after it. Lane-2's
            # norm is split: pairs 8-14 as soon as their absmax is in,
            # pair 15 separately, so only 2 J-writes trail the last pair.
            # Lane-1 chunks interleave only with DMA/gps/PE work — no DVE
            # ops are emitted between chunks, so the scan stream is never
            # head-of-line blocked. All late absmax batch after lane 1
            # (they are ~4us and their matmuls are long done by then).
            t_emitted = 0
            for p in range(8, NPAIR):
                pss[p] = do_pool(p)
                scan_lane(0, t_emitted, t_emitted + LANE1_CHUNK)
                t_emitted += LANE1_CHUNK
            if t_emitted < TB:
                scan_lane(0, t_emitted, TB)
            for p in range(8, NPAIR):
                do_absmax(p, pss[p])
            lane_norm(1, pss, 0, 16)
            lane_out(0)
            scan_lane(1)
            lane_out(1)

    nc.compile()
    return nc


class _Runner:
    """Compiles the Bass program once and executes it via PJRT shard_map
    across the 8 NeuronCores (mirrors bass2jax.run_bass_via_pjrt, but
    keeps the jitted callable for cheap repeat calls)."""

    def __init__(self):
        import jax
        import numpy as _np
        from jax.sharding import Mesh, PartitionSpec
        from jax.experimental.shard_map import shard_map
        import concourse.mybir as mybir
        from concourse.bass2jax import (_bass_exec_p, install_neuronx_cc_hook,
                                        partition_id_tensor)

        install_neuronx_cc_hook()
        nc = _build_nc()
        self.nc = nc

        partition_name = (nc.partition_id_tensor.name
                          if nc.partition_id_tensor else None)
        in_names, out_names, out_avals, zero_outs = [], [], [], []
        for alloc in nc.m.functions[0].allocations:
            if not isinstance(alloc, mybir.MemoryLocationSet):
                continue
            name = alloc.memorylocations[0].name
            if alloc.kind == "ExternalInput":
                if name != partition_name:
                    in_names.append(name)
            elif alloc.kind == "ExternalOutput":
                out_names.append(name)
                shape = tuple(alloc.tensor_shape)
                dtype = mybir.dt.np(alloc.dtype)
                out_avals.append(jax.core.ShapedArray(shape, dtype))
                zero_outs.append(_np.zeros(shape, dtype))
        self.in_names = list(in_names)
        self.out_names = out_names
        n_params = len(in_names)
        all_in_names = in_names + out_names
        if partition_name is not None:
            all_in_names.append(partition_name)

        def _body(*args):
            operands = list(args)
            if partition_name is not None:
                operands.append(partition_id_tensor())
            outs = _bass_exec_p.bind(
                *operands,
                out_avals=tuple(out_avals),
                in_names=tuple(all_in_names),
                out_names=tuple(out_names),
                lowering_input_output_aliases=(),
                sim_require_finite=True,
                sim_require_nnan=True,
                nc=nc,
            )
            return tuple(outs)

        devices = jax.devices()[:NCORES]
        self.mesh = Mesh(np.asarray(devices), ("core",))
        in_specs = (PartitionSpec("core"),) * (n_params + len(out_names))
        out_specs = (PartitionSpec("core"),) * len(out_names)
        self.fn = jax.jit(shard_map(_body, mesh=self.mesh, in_specs=in_specs,
                                    out_specs=out_specs, check_rep=False),
                          keep_unused=True)
        self.zero_outs = zero_outs
        self.out_avals = out_avals

    def concat_inputs(self, per_core_maps):
        return [np.concatenate([m[name] for m in per_core_maps], axis=0)
                for name in self.in_names]

    def run(self, concat_in):
        concat_zeros = [np.zeros((NCORES * z.shape[0], *z.shape[1:]), z.dtype)
                        for z in self.zero_outs]
        out_arrs = self.fn(*concat_in, *concat_zeros)
        return [np.asarray(a) for a in out_arrs]


def _get_runner():
    global _runner
    if _runner is None:
        _runner = _Runner()
    return _runner


def kernel(**inputs):
    ts = np.ascontiguousarray(np.asarray(inputs["transmit_spikes"], np.float32))
    rs = np.ascontiguousarray(np.asarray(inputs["receive_spikes"], np.float32))
    wr, wt, scal, biasn = _build_consts(
        np.asarray(inputs["conv_w"]), np.asarray(inputs["conv_b"]),
        np.asarray(inputs["frequency"]), np.asarray(inputs["decay"]),
        np.asarray(inputs["threshold"]),
        np.asarray(inputs["transmit_weight"]),
        np.asarray(inputs["receive_weight"]))

    runner = _get_runner()
    per_core = []
    for cidx in range(NCORES):
        bsl = slice(cidx * BS, (cidx + 1) * BS)
        per_core.append({
            "receive": rs[bsl], "transmit": ts[bsl],
            "wr": wr, "wt": wt, "scal": scal, "biasn": biasn,
        })
    concat_in = runner.concat_inputs(per_core)
    outs = runner.run(concat_in)
    # output "out": (8*2, 16, 16) -> (core, lane, r, b_local);
    # batch b = 32*core + 16*lane + b_local, value row = r
    o = outs[0].reshape(NCORES, 2, 16, 16)
    full = np.transpose(o, (0, 1, 3, 2)).reshape(B, R)
    return np.ascontiguousarray(full.astype(np.float32))
